# revision 1
# baseline (speedup 1.0000x reference)
"""Dinov3 self-attention Bass kernel for TRN2.

Sharding: data-parallel over batch. B=8 batch elements -> 8 NeuronCores,
one full attention per core, weights replicated. No collectives.

Per-core layout strategy (all matmuls bf16 x bf16 -> fp32 PSUM):
  xT  [h, s]   : x cast to bf16, DMA-transposed           (h on partitions)
  WqT/WkT/WvT/WpT [h, o] : weights cast + DMA-transposed
  qT/kT [o, s] : projections computed transposed, RoPE'd
  v    [s, o]  : projection computed natural (M = s)
  scores.T [j, i] = kT^T @ qT per head (K=d=64, two heads row-packed)
  expS.T = exp(0.125 * scores.T) on ScalarE, psum -> sbuf bf16
  PV: ctx_u.T[d, i] = (v_h | ones)^T @ expS.T  -> row 64 = softmax denominator
  normalize: ctxT = ctx_u.T * bcast(1/denom)   (DVE + DMA partition-broadcast)
  out[i, o] = ctxT^T @ WpT (+ ones x bp)       -> fp32 -> DRAM
"""

import contextlib
import os
import sys

import numpy as np

sys.path.insert(0, "/opt/trn_rl_repo")

import concourse.bacc as bacc
import concourse.bass as bass
import concourse.tile as tile
from concourse import mybir

S = 1374
H = 768
NH = 12
D = 64
NROT = 1369
PREFIX = S - NROT  # 5
B = 8

P = 128
NSTILE = (S + P - 1) // P  # 11 s-tiles, last has 94 rows
NOTILE = H // P  # 6
SPAD = NSTILE * P  # 1408
ICHUNKS = ((0, 687), (687, 687))  # i-chunks, 2 psum banks each
NCHUNK = len(ICHUNKS)
BANK = 512  # fp32 elements per psum bank (matmul N limit)
SCR_W = 768  # padded width of the denominator scratch rows

F32 = mybir.dt.float32
BF16 = mybir.dt.bfloat16


def _subchunks(total):
    """Split a free-dim range into <=BANK pieces aligned to bank boundaries."""
    out = []
    off = 0
    while off < total:
        n = min(BANK, total - off)
        out.append((off, n))
        off += n
    return out


def _stile(i):
    """(start, size) of s-tile i."""
    start = i * P
    return start, min(P, S - start)


def build_kernel(nc):
    x_ext = nc.declare_dram_parameter("hidden_states", [S, H], F32, isOutput=False)
    sin_ext = nc.declare_dram_parameter("sin", [NROT, D], F32, isOutput=False)
    cos_ext = nc.declare_dram_parameter("cos", [NROT, D], F32, isOutput=False)
    wq_ext = nc.declare_dram_parameter("Wq", [H, H], F32, isOutput=False)
    bq_ext = nc.declare_dram_parameter("bq", [H], F32, isOutput=False)
    wk_ext = nc.declare_dram_parameter("Wk", [H, H], F32, isOutput=False)
    wv_ext = nc.declare_dram_parameter("Wv", [H, H], F32, isOutput=False)
    bv_ext = nc.declare_dram_parameter("bv", [H], F32, isOutput=False)
    wp_ext = nc.declare_dram_parameter("Wp", [H, H], F32, isOutput=False)
    bp_ext = nc.declare_dram_parameter("bp", [H], F32, isOutput=False)
    out_ext = nc.declare_dram_parameter("out", [S, H], F32, isOutput=True)

    with tile.TileContext(nc) as tc:
        _body(tc, x_ext, sin_ext, cos_ext, wq_ext, bq_ext, wk_ext,
              wv_ext, bv_ext, wp_ext, bp_ext, out_ext)
    nc.compile()
    return nc


def _body(tc, x_ext, sin_ext, cos_ext, wq_ext, bq_ext, wk_ext, wv_ext,
          bv_ext, wp_ext, bp_ext, out_ext):
    nc = tc.nc
    from concourse.masks import make_identity

    with contextlib.ExitStack() as ctx:
        # ---------------- long-lived pools ----------------
        persist = ctx.enter_context(tc.tile_pool(name="persist", bufs=1))
        psum_qk = ctx.enter_context(tc.tile_pool(name="psum_qk", bufs=2, space="PSUM"))
        psum_pv = ctx.enter_context(tc.tile_pool(name="psum_pv", bufs=2, space="PSUM"))

        xT = persist.tile([P, NOTILE, SPAD], BF16)     # xT[p, t, s] = x[s, 128t+p]
        wpT = persist.tile([P, NOTILE, H], BF16)
        qT = persist.tile([P, NOTILE, SPAD], BF16)     # roped q, [o, s] layout
        kT = persist.tile([P, NOTILE, SPAD], BF16)
        # v[s, (h, d|1)]: per head 64 v columns + a ones column, so the PV
        # matmul computes ctx rows AND the softmax denominator in one M=65 MM
        vsb = persist.tile([P, NSTILE, NH, D + 1], BF16)
        ctxT = persist.tile([P, NOTILE, SPAD], BF16)   # ctx^T [(h,d), i]
        cc2 = persist.tile([P, SPAD], BF16)            # cos^T stacked twice
        ss2 = persist.tile([P, SPAD], BF16)            # sin^T stacked, sign-baked
        bq_sb = persist.tile([P, NOTILE], F32)
        bv_row = persist.tile([1, H], BF16)
        bp_row = persist.tile([1, H], BF16)
        ones_row = persist.tile([1, P], BF16)          # K=1 bias matmuls (lhsT)

        nc.vector.memset(ones_row, 1.0)
        # ones columns of vsb (index 64 of each head's slot)
        nc.vector.memset(vsb[:, :, :, D:D + 1], 1.0)

        with tc.tile_pool(name="wqkv", bufs=1) as wqkv_pool, \
             tc.tile_pool(name="ropet", bufs=3) as ropet, \
             tc.tile_pool(name="setup_stage", bufs=2) as stage:
            wqT = wqkv_pool.tile([P, NOTILE, H], BF16)
            wkT = wqkv_pool.tile([P, NOTILE, H], BF16)
            wvT = wqkv_pool.tile([P, NOTILE, H], BF16)

            # ---------------- biases ----------------
            # bq as [128, 6]: column t = bq[128t : 128t+128]
            nc.sync.dma_start(out=bq_sb,
                              in_=bq_ext.rearrange("(t p) -> p t", p=P))
            bstage = stage.tile([1, H], F32, tag="bias_stage", bufs=1)
            nc.sync.dma_start(out=bstage,
                              in_=bv_ext.rearrange("(a h) -> a h", a=1))
            nc.vector.tensor_copy(out=bv_row, in_=bstage)
            bstage2 = stage.tile([1, H], F32, tag="bias_stage2", bufs=1)
            nc.sync.dma_start(out=bstage2,
                              in_=bp_ext.rearrange("(a h) -> a h", a=1))
            nc.vector.tensor_copy(out=bp_row, in_=bstage2)

            def load_weight(w_ext, wT):
                for r in range(NOTILE):  # row tile of W (o dim)
                    ws = stage.tile([P, H], F32, tag="w_stage", name=f"ws_{r}")
                    wb = stage.tile([P, H], BF16, tag="w_stage_bf", name=f"wb_{r}")
                    nc.sync.dma_start(out=ws, in_=w_ext[r * P:(r + 1) * P, :])
                    nc.vector.tensor_copy(out=wb, in_=ws)
                    nc.scalar.dma_start_transpose(
                        out=wT[:, :, r * P:(r + 1) * P], in_=wb)

            def qk_proj(wT, dstT, bias):
                for ot in range(NOTILE):
                    qb = ropet.tile([P, SPAD], BF16, tag="qb", name=f"qb_{ot}")
                    for (i0, ilen) in ICHUNKS:
                        ps = psum_qk.tile([P, 2 * BANK], F32, tag="qk",
                                          name="qkps")[:, :ilen]
                        for kt in range(NOTILE):
                            for (o, n) in _subchunks(ilen):
                                nc.tensor.matmul(
                                    ps[:, o:o + n],
                                    wT[:, kt, ot * P:(ot + 1) * P],
                                    xT[:, kt, i0 + o:i0 + o + n],
                                    start=(kt == 0), stop=(kt == NOTILE - 1))
                        # evict + bias (per-partition scalar) -> bf16 on ACT
                        # (idle during the projection phase)
                        if bias:
                            nc.scalar.add(qb[:, i0:i0 + ilen], ps,
                                          bq_sb[:, ot:ot + 1])
                        else:
                            nc.scalar.copy(out=qb[:, i0:i0 + ilen], in_=ps)
                    # RoPE: rot[p] = qb[pair(p)] via partition-shifted DMA
                    rot = ropet.tile([P, NROT], BF16, tag="rot", name=f"rot_{ot}")
                    for (dst0, src0) in ((0, 32), (32, 0), (64, 96), (96, 64)):
                        nc.sync.dma_start(
                            out=rot[dst0:dst0 + 32, :],
                            in_=qb[src0:src0 + 32, PREFIX:PREFIX + NROT])
                    sl = slice(PREFIX, PREFIX + NROT)
                    nc.vector.tensor_mul(dstT[:, ot, sl], qb[:, sl],
                                         cc2[:, :NROT])
                    nc.vector.tensor_mul(rot, rot, ss2[:, :NROT])
                    nc.vector.tensor_add(dstT[:, ot, sl], dstT[:, ot, sl], rot)
                    nc.vector.tensor_copy(out=dstT[:, ot, 0:PREFIX],
                                          in_=qb[:, 0:PREFIX])

            # emission order = DMA queue order: Wq, x, (q-proj), Wk, (k-proj),
            # Wv, (v-proj), Wp -- gets the PE going as early as possible
            load_weight(wq_ext, wqT)

            # ---------------- load & transpose x ----------------
            for st in range(NSTILE):
                s0, ssz = _stile(st)
                xs = stage.tile([P, H], F32, tag="x_stage", name=f"xs_{st}")
                xb = stage.tile([P, H], BF16, tag="x_stage_bf", name=f"xb_{st}")
                if ssz < P:
                    nc.vector.memset(xb, 0.0)
                nc.sync.dma_start(out=xs[:ssz], in_=x_ext[s0:s0 + ssz, :])
                nc.vector.tensor_copy(out=xb[:ssz], in_=xs[:ssz])
                nc.scalar.dma_start_transpose(out=xT[:, :, s0:s0 + P], in_=xb)

            # ---------------- sin/cos tables ----------------
            # [NROT, 64] f32 -> bf16 [64, s]: stack 11 row-tiles side by side
            # (padded to 128 cols each) and transpose them all with ONE
            # batched xbar DMA, then copy into both halves of [128, s].
            n_rtile = (NROT + P - 1) // P
            for src_ext, dstT in ((cos_ext, cc2), (sin_ext, ss2)):
                cst_all = stage.tile([P, SPAD], F32, tag="cs_stage")
                csb_all = stage.tile([P, SPAD], BF16, tag="cs_stage_bf")
                csT3 = stage.tile([P, n_rtile, P], BF16, tag="cs_T3")
                nc.vector.memset(csb_all, 0.0)
                for i in range(n_rtile):
                    r0 = i * P
                    rsz = min(P, NROT - r0)
                    nc.sync.dma_start(out=cst_all[:rsz, i * P:i * P + D],
                                      in_=src_ext[r0:r0 + rsz, :])
                    nc.vector.tensor_copy(
                        out=csb_all[:rsz, i * P:i * P + D],
                        in_=cst_all[:rsz, i * P:i * P + D])
                nc.scalar.dma_start_transpose(out=csT3, in_=csb_all)
                for i in range(n_rtile):
                    r0 = i * P
                    rsz = min(P, NROT - r0)
                    for half in range(2):
                        nc.scalar.copy(
                            out=dstT[64 * half:64 * half + 64, r0:r0 + rsz],
                            in_=csT3[0:D, i, :rsz])
            # bake rotate_half sign into ss2: rows 0:32 and 64:96 negated
            for base in (0, 64):
                sl = slice(base, base + 32)
                nc.vector.tensor_scalar_mul(ss2[sl, :NROT],
                                            ss2[sl, :NROT], -1.0)


            qk_proj(wqT, qT, True)
            load_weight(wk_ext, wkT)
            qk_proj(wkT, kT, False)
            load_weight(wv_ext, wvT)

            # ---------------- v projection (natural out) ----------------
            for st in range(NSTILE):
                s0, ssz = _stile(st)
                for ci, (o, n) in enumerate(_subchunks(H)):
                    ps = psum_pv.tile([P, 2 * BANK], F32, tag="pv",
                                      name=f"vps_{st}_{ci}")[:, :n]
                    for kt in range(NOTILE):
                        nc.tensor.matmul(
                            ps[:ssz, :],
                            xT[:, kt, s0:s0 + ssz],
                            wvT[:, kt, o:o + n],
                            start=(kt == 0), stop=False)
                    # bias: += ones[s] x bv[o]  (K=1 rank-1 update ends group)
                    nc.tensor.matmul(
                        ps[:ssz, :],
                        ones_row[:, :ssz],
                        bv_row[:, o:o + n],
                        start=False, stop=True)
                    # scatter heads into their 65-wide slots (8 heads per 512)
                    nc.scalar.copy(
                        out=vsb[:ssz, st, o // D:(o + n) // D, 0:D],
                        in_=ps[:ssz, :].rearrange("p (h d) -> p h d", d=D))

            load_weight(wp_ext, wpT)

        # ---------------- attention (6 head pairs) ----------------
        exps_pool = ctx.enter_context(tc.tile_pool(name="exps_pool", bufs=6))
        norm_pool = ctx.enter_context(tc.tile_pool(name="norm_pool", bufs=6))
        outst = ctx.enter_context(tc.tile_pool(name="outst", bufs=2))
        dram_pool = ctx.enter_context(
            tc.tile_pool(name="dram_pool", bufs=1, space="DRAM"))
        rs_scratch = dram_pool.tile([NH * NCHUNK, SCR_W], F32)  # 36 rows
        # prefill with 1.0 so the 687:768 pad cols stay finite under recip
        ones_f32 = norm_pool.tile([1, SCR_W], F32, tag="ones_f32", bufs=1)
        nc.vector.memset(ones_f32, 1.0)
        for idx in range(NH * NCHUNK):
            nc.sync.dma_start(out=rs_scratch[idx:idx + 1, :], in_=ones_f32)

        for pt in range(NOTILE):  # head pair = heads (2pt, 2pt+1)
            for c, (i0, ilen) in enumerate(ICHUNKS):
                pv_ps = []
                for hh in range(2):
                    pv_ps.append(psum_pv.tile([P, 2 * BANK], F32, tag="pv",
                                              name=f"pvps_{pt}_{c}_{hh}")[:, :ilen])
                for jt in range(NSTILE):
                    j0, jsz = _stile(jt)
                    exps = []
                    for hh in range(2):  # head half: partitions 64*hh
                        hb = 64 * hh
                        sc = psum_qk.tile([P, 2 * BANK], F32, tag="qk",
                                          name=f"scps_{pt}_{c}_{jt}_{hh}")[:, :ilen]
                        for (o, n) in _subchunks(ilen):
                            nc.tensor.matmul(
                                sc[:jsz, o:o + n],
                                kT[hb:hb + 64, pt, j0:j0 + jsz],
                                qT[hb:hb + 64, pt, i0 + o:i0 + o + n],
                                start=True, stop=True)
                        es = exps_pool.tile([P, 2 * BANK], BF16, tag="es",
                                            name=f"es_{pt}_{c}_{jt}_{hh}")
                        nc.scalar.activation(
                            out=es[:jsz, :ilen], in_=sc[:jsz, :],
                            func=mybir.ActivationFunctionType.Exp,
                            scale=float(D) ** -0.5)
                        exps.append(es)
                    for hh in range(2):
                        h = 2 * pt + hh
                        es = exps[hh]
                        # ctx_u^T rows 0:64 + denominator row 64, one MM
                        for (o, n) in _subchunks(ilen):
                            nc.tensor.matmul(
                                pv_ps[hh][0:D + 1, o:o + n],
                                vsb[:jsz, jt, h, :],
                                es[:jsz, o:o + n],
                                start=(jt == 0), stop=(jt == NSTILE - 1))
                # normalize: ctxT = ctx_u^T * (1/denom), bcast over partitions.
                # 1. evict psum to sbuf immediately (frees the pv psum slot).
                # 2. reciprocal is free-dim-serial (~8 cyc/elem), so reshape
                #    the denom row into [128, 6] via a DRAM bounce and run
                #    the recip across partitions instead.
                # 3. SBUF APs can't have partition step 0, DRAM APs can --
                #    broadcast-read the recip'd row from the DRAM scratch.
                for hh in range(2):
                    idx = (pt * 2 + hh) * NCHUNK + c
                    stg = norm_pool.tile([D + 1, 2 * BANK], F32, tag="stg",
                                         name=f"stg_{pt}_{c}_{hh}")[:, :ilen]
                    nc.vector.tensor_copy(out=stg, in_=pv_ps[hh][0:D + 1, :])
                    nc.sync.dma_start(out=rs_scratch[idx:idx + 1, :ilen],
                                      in_=stg[D:D + 1, :])
                    rsh = norm_pool.tile([P, SCR_W // P], F32, tag="rsh",
                                         name=f"rsh_{pt}_{c}_{hh}")
                    nc.sync.dma_start(
                        out=rsh, in_=rs_scratch[idx, :].rearrange(
                            "(i p) -> p i", p=P))
                    nc.vector.reciprocal(out=rsh, in_=rsh)
                    nc.sync.dma_start(
                        out=rs_scratch[idx, :].rearrange("(i p) -> p i", p=P),
                        in_=rsh)
                    bc = norm_pool.tile([D, 2 * BANK], F32, tag="bc",
                                        name=f"bc_{pt}_{c}_{hh}")[:, :ilen]
                    scr_row = rs_scratch[idx:idx + 1, :ilen]
                    bcast_src = bass.AP(
                        tensor=scr_row.tensor, offset=scr_row.offset,
                        ap=[[0, D]] + list(scr_row.ap[1:]))
                    nc.sync.dma_start(out=bc, in_=bcast_src)
                    nc.vector.tensor_mul(
                        ctxT[64 * hh:64 * hh + 64, pt, i0:i0 + ilen],
                        stg[0:D, :], bc)

        # ---------------- output projection ----------------
        for it in range(NSTILE):
            s0, ssz = _stile(it)
            ot = outst.tile([P, H], F32, tag="ostage", name=f"ost_{it}")
            for ci, (o, n) in enumerate(_subchunks(H)):
                ps = psum_pv.tile([P, 2 * BANK], F32, tag="pv",
                                  name=f"ops_{it}_{ci}")[:, :n]
                for kt in range(NOTILE):
                    nc.tensor.matmul(
                        ps[:ssz, :],
                        ctxT[:, kt, s0:s0 + ssz],
                        wpT[:, kt, o:o + n],
                        start=(kt == 0), stop=False)
                nc.tensor.matmul(
                    ps[:ssz, :],
                    ones_row[:, :ssz],
                    bp_row[:, o:o + n],
                    start=False, stop=True)
                nc.scalar.copy(out=ot[:ssz, o:o + n], in_=ps[:ssz])
            nc.sync.dma_start(out=out_ext[s0:s0 + ssz, :], in_=ot[:ssz])


_NC_CACHE = None


def get_nc():
    global _NC_CACHE
    if _NC_CACHE is None:
        nc = bacc.Bacc(None, target_bir_lowering=False, debug=False)
        _NC_CACHE = build_kernel(nc)
    return _NC_CACHE


def kernel(**inputs):
    from concourse.bass_utils import run_bass_kernel_spmd

    nc = get_nc()
    names = ["hidden_states", "sin", "cos", "Wq", "bq", "Wk", "Wv", "bv", "Wp", "bp"]
    arrs = {k: np.ascontiguousarray(np.asarray(inputs[k], dtype=np.float32))
            for k in names}
    in_maps = []
    for b in range(B):
        m = {k: arrs[k] for k in names if k != "hidden_states"}
        m["hidden_states"] = np.ascontiguousarray(arrs["hidden_states"][b])
        in_maps.append(m)
    res = run_bass_kernel_spmd(nc, in_maps, core_ids=list(range(B)))
    out = np.stack([res.results[b]["out"] for b in range(B)], axis=0)
    return out.astype(np.float32)


if __name__ == "__main__":
    # quick smoke: build only
    nc = get_nc()
    print("built ok")



# revision 2
# speedup vs baseline: 1.0496x; 1.0496x over previous
"""Dinov3 self-attention Bass kernel for TRN2.

Sharding: data-parallel over batch. B=8 batch elements -> 8 NeuronCores,
one full attention per core, weights replicated. No collectives.

Per-core structure (all matmuls bf16 x bf16 -> fp32 PSUM):
  xT   [h, s]  : x cast bf16, DMA-transposed (h on partitions)
  w*T  [h, o]  : weights cast + DMA-transposed
  projections run in NATURAL layout: q[i, o] = x @ Wq^T (+ ones x bq),
    RoPE applied on DVE along the free dim (rotate_half is a free-dim
    shuffle there), then per-tile DMA-transpose into qT/kT [o, s].
  v natural -> vsb[j, jt, h, 65] with a ones column per head (PV matmul
    computes ctx rows AND the softmax denominator in one M=65 matmul).
  scores^T[j, i] per head = kT_h^T @ qT_h (K=64); the two heads of a
    128-partition block run CONCURRENTLY on disjoint PE row strips.
  exp on ACT evicts TWO j-tiles per instruction ([128, 1024] psum read)
    to amortize the 352-cycle ACT overhead.
  PV: ctx_u^T[d|den, i] += vsb_h^T @ es_h, accumulated over all j tiles.
  normalize: recip(denoms) via DRAM partition-reshape bounce, then
    DVE mul (psum x bcast) -> ctxT bf16.
  out[i, o] = ctx^T^T @ WpT (+ ones x bp) -> fp32 -> DRAM.
  i-chunk is the OUTER attention loop so out-proj overlaps attention.
"""

import contextlib
import sys

import numpy as np

sys.path.insert(0, "/opt/trn_rl_repo")

import concourse.bacc as bacc
import concourse.bass as bass
import concourse.tile as tile
from concourse import mybir

S = 1374
H = 768
NH = 12
D = 64
NROT = 1369
PREFIX = S - NROT  # 5
B = 8

P = 128
NT = (S + P - 1) // P   # 11 s-tiles, last has 94 rows
KT = H // P             # 6 contraction blocks
SPAD = NT * P           # 1408
ICH = ((0, 512), (512, 512), (1024, 350))  # i-chunks
SCR_W = 512             # denominator scratch row width

F32 = mybir.dt.float32
BF16 = mybir.dt.bfloat16


def _stile(i):
    start = i * P
    return start, min(P, S - start)


def _nchunks(total, width=512):
    out, off = [], 0
    while off < total:
        n = min(width, total - off)
        out.append((off, n))
        off += n
    return out


def _bcast_mid(ap2d, reps):
    """[p, f] AP -> [p, reps, f] with a 0-step replicated middle dim."""
    return bass.AP(tensor=ap2d.tensor, offset=ap2d.offset,
                   ap=[list(ap2d.ap[0]), [0, reps]] + [list(a) for a in ap2d.ap[1:]])


def build_kernel(nc):
    x_ext = nc.declare_dram_parameter("hidden_states", [S, H], F32, isOutput=False)
    sin_ext = nc.declare_dram_parameter("sin", [NROT, D], F32, isOutput=False)
    cos_ext = nc.declare_dram_parameter("cos", [NROT, D], F32, isOutput=False)
    wq_ext = nc.declare_dram_parameter("Wq", [H, H], F32, isOutput=False)
    bq_ext = nc.declare_dram_parameter("bq", [H], F32, isOutput=False)
    wk_ext = nc.declare_dram_parameter("Wk", [H, H], F32, isOutput=False)
    wv_ext = nc.declare_dram_parameter("Wv", [H, H], F32, isOutput=False)
    bv_ext = nc.declare_dram_parameter("bv", [H], F32, isOutput=False)
    wp_ext = nc.declare_dram_parameter("Wp", [H, H], F32, isOutput=False)
    bp_ext = nc.declare_dram_parameter("bp", [H], F32, isOutput=False)
    out_ext = nc.declare_dram_parameter("out", [S, H], F32, isOutput=True)

    with tile.TileContext(nc) as tc:
        _body(tc, x_ext, sin_ext, cos_ext, wq_ext, bq_ext, wk_ext,
              wv_ext, bv_ext, wp_ext, bp_ext, out_ext)
    nc.compile()
    return nc


def _body(tc, x_ext, sin_ext, cos_ext, wq_ext, bq_ext, wk_ext, wv_ext,
          bv_ext, wp_ext, bp_ext, out_ext):
    nc = tc.nc

    with contextlib.ExitStack() as ctx:
        persist = ctx.enter_context(tc.tile_pool(name="persist", bufs=1))
        # PSUM: 2x [128,1024] (4 banks) + 4x [128,512] (4 banks) = 8 banks
        ps_big = ctx.enter_context(tc.tile_pool(name="ps_big", bufs=2, space="PSUM"))
        ps_pv = ctx.enter_context(tc.tile_pool(name="ps_pv", bufs=4, space="PSUM"))

        xT = persist.tile([P, KT, SPAD], BF16)
        wvT = persist.tile([P, KT, H], BF16)
        wpT = persist.tile([P, KT, H], BF16)
        qT = persist.tile([P, KT, SPAD], BF16)
        kT = persist.tile([P, KT, SPAD], BF16)
        ctxT = persist.tile([P, KT, SPAD], BF16)
        vsb = persist.tile([P, NT, NH, D + 1], BF16)
        ccos = persist.tile([P, NT, D], BF16)   # cos, i-tile aligned (prefix rows = 1)
        ssin = persist.tile([P, NT, D], BF16)   # sin, rotate-half sign baked in cols 0:32
        bq_row = persist.tile([1, H], BF16)
        bv_row = persist.tile([1, H], BF16)
        bp_row = persist.tile([1, H], BF16)
        ones_row = persist.tile([1, P], BF16)

        nc.vector.memset(ones_row, 1.0)
        nc.vector.memset(vsb[:, :, :, D:D + 1], 1.0)
        nc.vector.memset(ctxT[:, :, S:SPAD], 0.0)

        # preload the exp table set so the first real exp doesn't stall
        with tc.tile_pool(name="warm", bufs=1) as warm:
            wtile = warm.tile([1, 2], F32)
            nc.vector.memset(wtile, 0.0)
            nc.scalar.activation(out=wtile[:, 1:2], in_=wtile[:, 0:1],
                                 func=mybir.ActivationFunctionType.Exp)

        with tc.tile_pool(name="wqk", bufs=1) as wqk_pool, \
             tc.tile_pool(name="stage", bufs=3) as stage, \
             tc.tile_pool(name="natq", bufs=3) as natq, \
             tc.tile_pool(name="rope", bufs=4) as rope:
            wqT = wqk_pool.tile([P, KT, H], BF16)
            wkT = wqk_pool.tile([P, KT, H], BF16)

            # ---------------- biases ----------------
            for b_ext, b_row in ((bq_ext, bq_row), (bv_ext, bv_row),
                                 (bp_ext, bp_row)):
                bs = stage.tile([1, H], F32, tag="bias_stage")
                nc.sync.dma_start(out=bs, in_=b_ext.rearrange("(a h) -> a h", a=1))
                nc.vector.tensor_copy(out=b_row, in_=bs)

            def load_weight(w_ext, wT):
                for r in range(KT):
                    ws = stage.tile([P, H], F32, tag="w_stage", name=f"ws_{r}")
                    wb = stage.tile([P, H], BF16, tag="w_stage_bf", name=f"wb_{r}")
                    nc.sync.dma_start(out=ws, in_=w_ext[r * P:(r + 1) * P, :])
                    nc.vector.tensor_copy(out=wb, in_=ws)
                    nc.scalar.dma_start_transpose(
                        out=wT[:, :, r * P:(r + 1) * P], in_=wb)

            load_weight(wq_ext, wqT)

            # ---------------- x load & transpose ----------------
            for st in range(NT):
                s0, ssz = _stile(st)
                xs = stage.tile([P, H], F32, tag="x_stage", name=f"xs_{st}")
                xb = stage.tile([P, H], BF16, tag="x_stage_bf", name=f"xb_{st}")
                if ssz < P:
                    nc.vector.memset(xb, 0.0)
                nc.sync.dma_start(out=xs[:ssz], in_=x_ext[s0:s0 + ssz, :])
                nc.vector.tensor_copy(out=xb[:ssz], in_=xs[:ssz])
                nc.scalar.dma_start_transpose(out=xT[:, :, s0:s0 + P], in_=xb)

            # ---------------- sin/cos tables (i-tile aligned) ----------------
            cstg = stage.tile([P, NT, D], F32, tag="cos_stage")
            sstg = stage.tile([P, NT, D], F32, tag="sin_stage")
            nc.vector.memset(cstg, 1.0)
            nc.vector.memset(sstg, 0.0)
            for st in range(NT):
                if st == 0:
                    nc.sync.dma_start(out=cstg[PREFIX:P, 0, :],
                                      in_=cos_ext[0:P - PREFIX, :])
                    nc.sync.dma_start(out=sstg[PREFIX:P, 0, :],
                                      in_=sin_ext[0:P - PREFIX, :])
                else:
                    r0 = st * P - PREFIX
                    rsz = min(P, NROT - r0)
                    nc.sync.dma_start(out=cstg[:rsz, st, :],
                                      in_=cos_ext[r0:r0 + rsz, :])
                    nc.sync.dma_start(out=sstg[:rsz, st, :],
                                      in_=sin_ext[r0:r0 + rsz, :])
            nc.vector.tensor_copy(out=ccos, in_=cstg)
            nc.vector.tensor_copy(out=ssin, in_=sstg)
            # bake rotate_half sign: first-half sin columns negated
            nc.vector.tensor_scalar_mul(ssin[:, :, 0:32], ssin[:, :, 0:32], -1.0)

            # ---------------- q/k projection + RoPE ----------------
            def qk_proj(wT, dstT, bias_row):
                for st in range(NT):
                    s0, _ = _stile(st)
                    ps = ps_big.tile([P, 1024], F32, tag="big",
                                     name=f"qk_{id(wT)}_{st}")[:, :H]
                    for kt in range(KT):
                        for (o, n) in _nchunks(H):
                            nc.tensor.matmul(
                                ps[:, o:o + n],
                                xT[:, kt, s0:s0 + P],
                                wT[:, kt, o:o + n],
                                start=(kt == 0),
                                stop=(kt == KT - 1 and bias_row is None))
                    if bias_row is not None:
                        for (o, n) in _nchunks(H):
                            nc.tensor.matmul(ps[:, o:o + n], ones_row,
                                             bias_row[:, o:o + n],
                                             start=False, stop=True)
                    qn = natq.tile([P, NH, D], BF16, tag="qn", name=f"qn_{st}")
                    nc.scalar.copy(out=qn,
                                   in_=ps.rearrange("p (h d) -> p h d", d=D))
                    rot = rope.tile([P, NH, D], BF16, tag="rot", name=f"rot_{st}")
                    qr = rope.tile([P, NH, D], BF16, tag="qr", name=f"qr_{st}")
                    nc.vector.tensor_mul(rot[:, :, 0:32], qn[:, :, 32:64],
                                         _bcast_mid(ssin[:, st, 0:32], NH))
                    nc.vector.tensor_mul(rot[:, :, 32:64], qn[:, :, 0:32],
                                         _bcast_mid(ssin[:, st, 32:64], NH))
                    nc.vector.tensor_mul(qr, qn, _bcast_mid(ccos[:, st, :], NH))
                    nc.vector.tensor_add(qr, qr, rot)
                    nc.scalar.dma_start_transpose(
                        out=dstT[:, :, s0:s0 + P],
                        in_=qr.rearrange("p h d -> p (h d)"))

            qk_proj(wqT, qT, bq_row)
            load_weight(wk_ext, wkT)
            qk_proj(wkT, kT, None)
            load_weight(wv_ext, wvT)

            # ---------------- v projection ----------------
            for st in range(NT):
                s0, ssz = _stile(st)
                ps = ps_big.tile([P, 1024], F32, tag="big", name=f"v_{st}")[:, :H]
                for kt in range(KT):
                    for (o, n) in _nchunks(H):
                        nc.tensor.matmul(ps[:ssz, o:o + n],
                                         xT[:, kt, s0:s0 + ssz],
                                         wvT[:, kt, o:o + n],
                                         start=(kt == 0), stop=False)
                for (o, n) in _nchunks(H):
                    nc.tensor.matmul(ps[:ssz, o:o + n], ones_row[:, :ssz],
                                     bv_row[:, o:o + n], start=False, stop=True)
                nc.scalar.copy(out=vsb[:ssz, st, :, 0:D],
                               in_=ps[:ssz, :].rearrange("p (h d) -> p h d", d=D))

            load_weight(wp_ext, wpT)

        # ---------------- attention ----------------
        es_pool = ctx.enter_context(tc.tile_pool(name="es_pool", bufs=3))
        norm_pool = ctx.enter_context(tc.tile_pool(name="norm_pool", bufs=3))
        outst = ctx.enter_context(tc.tile_pool(name="outst", bufs=2))
        dram_pool = ctx.enter_context(
            tc.tile_pool(name="dram_pool", bufs=1, space="DRAM"))
        rs_scratch = dram_pool.tile([NH * len(ICH), SCR_W], F32)
        ones_f32 = norm_pool.tile([NH * len(ICH), SCR_W], F32, tag="ones_f32",
                                  bufs=1)
        nc.vector.memset(ones_f32, 1.0)
        nc.sync.dma_start(out=rs_scratch, in_=ones_f32)

        scaling = float(D) ** -0.5
        NJP = (NT + 1) // 2  # j-tile pairs

        def out_proj(st):
            s0, ssz = _stile(st)
            ps = ps_big.tile([P, 1024], F32, tag="big", name=f"o_{st}")[:, :H]
            for kt in range(KT):
                for (o, n) in _nchunks(H):
                    nc.tensor.matmul(ps[:ssz, o:o + n],
                                     ctxT[:, kt, s0:s0 + ssz],
                                     wpT[:, kt, o:o + n],
                                     start=(kt == 0), stop=False)
            for (o, n) in _nchunks(H):
                nc.tensor.matmul(ps[:ssz, o:o + n], ones_row[:, :ssz],
                                 bp_row[:, o:o + n], start=False, stop=True)
            ot = outst.tile([P, H], F32, tag="ostage", name=f"ost_{st}")
            nc.scalar.copy(out=ot[:ssz], in_=ps[:ssz, :])
            nc.sync.dma_start(out=out_ext[s0:s0 + ssz, :], in_=ot[:ssz])

        done_itiles = 0
        for c, (i0, ilen) in enumerate(ICH):
            for pt in range(KT):
                heads = (2 * pt, 2 * pt + 1)
                pvs = [ps_pv.tile([P, 512], F32, tag="pv",
                                  name=f"pv_{c}_{pt}_{hh}")[:, :ilen]
                       for hh in range(2)]
                for jp in range(NJP):
                    jts = [t for t in (2 * jp, 2 * jp + 1) if t < NT]
                    scs = [ps_big.tile([P, 1024], F32, tag="big",
                                       name=f"sc_{c}_{pt}_{jp}_{hh}")
                           for hh in range(2)]
                    for a, jt in enumerate(jts):
                        j0, jsz = _stile(jt)
                        for hh in range(2):
                            hb = 64 * hh
                            nc.tensor.matmul(
                                scs[hh][0:jsz, 512 * a:512 * a + ilen],
                                kT[hb:hb + 64, pt, j0:j0 + jsz],
                                qT[hb:hb + 64, pt, i0:i0 + ilen],
                                start=True, stop=True,
                                tile_position=(hb, 0))
                    ess = []
                    for hh in range(2):
                        es = es_pool.tile([P, 1024], BF16, tag=f"es{hh}",
                                          name=f"es_{c}_{pt}_{jp}_{hh}")
                        if len(jts) == 2 and ilen == 512:
                            nc.scalar.activation(
                                out=es, in_=scs[hh],
                                func=mybir.ActivationFunctionType.Exp,
                                scale=scaling)
                        else:
                            for a, jt in enumerate(jts):
                                _, jsz = _stile(jt)
                                nc.scalar.activation(
                                    out=es[0:jsz, 512 * a:512 * a + ilen],
                                    in_=scs[hh][0:jsz, 512 * a:512 * a + ilen],
                                    func=mybir.ActivationFunctionType.Exp,
                                    scale=scaling)
                        ess.append(es)
                    for a, jt in enumerate(jts):
                        _, jsz = _stile(jt)
                        for hh in range(2):
                            nc.tensor.matmul(
                                pvs[hh][0:D + 1, :],
                                vsb[0:jsz, jt, heads[hh], :],
                                ess[hh][0:jsz, 512 * a:512 * a + ilen],
                                start=(jt == 0), stop=(jt == NT - 1))
                # normalize -> ctxT
                for hh in range(2):
                    idx = heads[hh] * len(ICH) + c
                    dn = norm_pool.tile([1, SCR_W], F32, tag="dn",
                                        name=f"dn_{c}_{pt}_{hh}")
                    nc.vector.tensor_copy(out=dn[:, :ilen],
                                          in_=pvs[hh][D:D + 1, :])
                    nc.sync.dma_start(out=rs_scratch[idx:idx + 1, :ilen],
                                      in_=dn[:, :ilen])
                    rsh = norm_pool.tile([P, SCR_W // P], F32, tag="rsh",
                                         name=f"rsh_{c}_{pt}_{hh}")
                    nc.sync.dma_start(
                        out=rsh,
                        in_=rs_scratch[idx, :].rearrange("(i p) -> p i", p=P))
                    nc.vector.reciprocal(out=rsh, in_=rsh)
                    nc.sync.dma_start(
                        out=rs_scratch[idx, :].rearrange("(i p) -> p i", p=P),
                        in_=rsh)
                    bc = norm_pool.tile([D, SCR_W], F32, tag="bc",
                                        name=f"bc_{c}_{pt}_{hh}")[:, :ilen]
                    scr_row = rs_scratch[idx:idx + 1, :ilen]
                    bcast_src = bass.AP(
                        tensor=scr_row.tensor, offset=scr_row.offset,
                        ap=[[0, D]] + [list(a) for a in scr_row.ap[1:]])
                    nc.sync.dma_start(out=bc, in_=bcast_src)
                    nc.vector.tensor_mul(
                        ctxT[64 * hh:64 * hh + 64, pt, i0:i0 + ilen],
                        pvs[hh][0:D, :], bc)
            # out-proj for i-tiles fully covered by chunks <= this one
            lim = (i0 + ilen) // P
            while done_itiles < lim:
                out_proj(done_itiles)
                done_itiles += 1
        while done_itiles < NT:
            out_proj(done_itiles)
            done_itiles += 1


_NC_CACHE = None


def get_nc():
    global _NC_CACHE
    if _NC_CACHE is None:
        nc = bacc.Bacc(None, target_bir_lowering=False, debug=False)
        _NC_CACHE = build_kernel(nc)
    return _NC_CACHE


def kernel(**inputs):
    from concourse.bass_utils import run_bass_kernel_spmd

    nc = get_nc()
    names = ["hidden_states", "sin", "cos", "Wq", "bq", "Wk", "Wv", "bv", "Wp", "bp"]
    arrs = {k: np.ascontiguousarray(np.asarray(inputs[k], dtype=np.float32))
            for k in names}
    in_maps = []
    for b in range(B):
        m = {k: arrs[k] for k in names if k != "hidden_states"}
        m["hidden_states"] = np.ascontiguousarray(arrs["hidden_states"][b])
        in_maps.append(m)
    res = run_bass_kernel_spmd(nc, in_maps, core_ids=list(range(B)))
    out = np.stack([res.results[b]["out"] for b in range(B)], axis=0)
    return out.astype(np.float32)


if __name__ == "__main__":
    nc = get_nc()
    print("built ok")


# revision 3
# speedup vs baseline: 1.3340x; 1.2710x over previous
"""Dinov3 self-attention Bass kernel for TRN2.

Sharding: data-parallel over batch. B=8 batch elements -> 8 NeuronCores,
one full attention per core, weights replicated. No collectives.

Per-core structure (all matmuls bf16 x bf16 -> fp32 PSUM):
  xT   [h, s]  : x cast bf16, DMA-transposed (h on partitions)
  w*T  [h, o]  : weights cast + DMA-transposed
  projections run in NATURAL layout: q[i, o] = x @ Wq^T (+ ones x bq).
    RoPE runs on DVE reading the psum directly (rotate_half is a
    free-dim shuffle in this layout), then a per-tile DMA-transpose
    builds qT/kT [o, s].  ScalarE's strict FIFO carries ONLY the
    transposes in this phase so nothing serializes behind it.
  v natural -> vsb[j, jt, h, 65] with a ones column per head (PV matmul
    computes ctx rows AND the softmax denominator in one M=65 matmul).
  scores^T[j, i]: one [128,1024] psum tile per (c,pt,jt) holds BOTH
    heads of block pt; the two score MMs run concurrently on disjoint
    PE row strips (K=64 each). Double-buffered so exp hides under PE.
  exp: one ACT per (c,pt,jt) reading [jsz, 1024] (both heads) from
    psum -> es bf16.  For DVE_EXP_PTS head pairs, head1 instead uses a
    one-op DVE exp2 bit-trick (Schraudolph in bf16 domain) to offload
    the ACT engine.
  PV: ctx_u^T[d|den, i] += vsb_h^T @ es_h, accumulated over all jt.
  normalize: recip_approx(denom row) then one DRAM bounce for the
    partition-broadcast; DVE mul (psum x bcast) -> ctxT bf16.
  out[i, o] = ctxT^T @ WpT (+ ones x bp) -> fp32 -> DRAM.  Out-proj for
    chunk c is emitted one pt-iteration into chunk c+1 so its MMs never
    head-block the PE queue while ctxT normalization finishes.
"""

import contextlib
import sys

import numpy as np

sys.path.insert(0, "/opt/trn_rl_repo")

import concourse.bacc as bacc
import concourse.bass as bass
import concourse.tile as tile
from concourse import mybir

S = 1374
H = 768
NH = 12
D = 64
NROT = 1369
PREFIX = S - NROT  # 5
B = 8

P = 128
NT = (S + P - 1) // P   # 11 s-tiles, last has 94 rows
KT = H // P             # 6 contraction blocks
SPAD = NT * P           # 1408
ICH = ((0, 512), (512, 512), (1024, 350))  # i-chunks
SCR_W = 512             # denominator scratch row width

# head pairs whose ODD head uses the DVE exp2 bit-trick instead of ACT exp
DVE_EXP_PTS = (0, 2, 4)
# exp(z) ~ bf16_bits(round(z*log2e*128 + 128*(127-sigma))), z = s/8
EXP_A = 16.0 * 1.4426950408889634          # 128 * log2(e) / 8
EXP_B = 128.0 * (127.0 - 0.058)

F32 = mybir.dt.float32
BF16 = mybir.dt.bfloat16
I16 = mybir.dt.int16

SCALING = float(D) ** -0.5


def _stile(i):
    start = i * P
    return start, min(P, S - start)


def _nchunks(total, width=512):
    out, off = [], 0
    while off < total:
        n = min(width, total - off)
        out.append((off, n))
        off += n
    return out


def _bcast_mid(ap2d, reps):
    """[p, f] AP -> [p, reps, f] with a 0-step replicated middle dim."""
    return bass.AP(tensor=ap2d.tensor, offset=ap2d.offset,
                   ap=[list(ap2d.ap[0]), [0, reps]] + [list(a) for a in ap2d.ap[1:]])


def build_kernel(nc):
    x_ext = nc.declare_dram_parameter("hidden_states", [S, H], F32, isOutput=False)
    sin_ext = nc.declare_dram_parameter("sin", [NROT, D], F32, isOutput=False)
    cos_ext = nc.declare_dram_parameter("cos", [NROT, D], F32, isOutput=False)
    wq_ext = nc.declare_dram_parameter("Wq", [H, H], F32, isOutput=False)
    bq_ext = nc.declare_dram_parameter("bq", [H], F32, isOutput=False)
    wk_ext = nc.declare_dram_parameter("Wk", [H, H], F32, isOutput=False)
    wv_ext = nc.declare_dram_parameter("Wv", [H, H], F32, isOutput=False)
    bv_ext = nc.declare_dram_parameter("bv", [H], F32, isOutput=False)
    wp_ext = nc.declare_dram_parameter("Wp", [H, H], F32, isOutput=False)
    bp_ext = nc.declare_dram_parameter("bp", [H], F32, isOutput=False)
    out_ext = nc.declare_dram_parameter("out", [S, H], F32, isOutput=True)

    with tile.TileContext(nc) as tc:
        _body(tc, x_ext, sin_ext, cos_ext, wq_ext, bq_ext, wk_ext,
              wv_ext, bv_ext, wp_ext, bp_ext, out_ext)
    nc.compile()
    return nc


def _body(tc, x_ext, sin_ext, cos_ext, wq_ext, bq_ext, wk_ext, wv_ext,
          bv_ext, wp_ext, bp_ext, out_ext):
    nc = tc.nc

    with contextlib.ExitStack() as ctx:
        persist = ctx.enter_context(tc.tile_pool(name="persist", bufs=1))
        # PSUM: sc pool 2x [128,1024] (4 banks) + pv pool 4x [128,512]
        # (4 banks) = 8 banks.  Projections use sc; out-proj uses pv.
        ps_sc = ctx.enter_context(tc.tile_pool(name="ps_sc", bufs=2, space="PSUM"))
        ps_pv = ctx.enter_context(tc.tile_pool(name="ps_pv", bufs=4, space="PSUM"))

        xT = persist.tile([P, KT, SPAD], BF16)
        wvT = persist.tile([P, KT, H], BF16)
        wpT = persist.tile([P, KT, H], BF16)
        qT = persist.tile([P, KT, SPAD], BF16)
        kT = persist.tile([P, KT, SPAD], BF16)
        ctxT = persist.tile([P, KT, SPAD], BF16)
        vsb = persist.tile([P, NT, NH, D + 1], BF16)
        ccos = persist.tile([P, NT, D], BF16)   # cos, i-tile aligned (prefix rows = 1)
        ssin = persist.tile([P, NT, D], BF16)   # sin, rotate-half sign baked in cols 0:32
        bq_row = persist.tile([1, H], BF16)
        bv_row = persist.tile([1, H], BF16)
        bp_row = persist.tile([1, H], BF16)
        ones_row = persist.tile([1, P], BF16)

        nc.vector.memset(ones_row, 1.0)
        nc.vector.memset(vsb[:, :, :, D:D + 1], 1.0)
        nc.vector.memset(ctxT[:, :, S:SPAD], 0.0)

        # preload the exp table set so the first real exp doesn't stall
        with tc.tile_pool(name="warm", bufs=1) as warm:
            wtile = warm.tile([1, 2], F32)
            nc.vector.memset(wtile, 0.0)
            nc.scalar.activation(out=wtile[:, 1:2], in_=wtile[:, 0:1],
                                 func=mybir.ActivationFunctionType.Exp)

        with tc.tile_pool(name="wqk", bufs=1) as wqk_pool, \
             tc.tile_pool(name="stage", bufs=3) as stage, \
             tc.tile_pool(name="rope", bufs=4) as rope:
            wqT = wqk_pool.tile([P, KT, H], BF16)
            wkT = wqk_pool.tile([P, KT, H], BF16)

            # ---------------- biases ----------------
            for b_ext, b_row in ((bq_ext, bq_row), (bv_ext, bv_row),
                                 (bp_ext, bp_row)):
                bs = stage.tile([1, H], F32, tag="bias_stage")
                nc.sync.dma_start(out=bs, in_=b_ext.rearrange("(a h) -> a h", a=1))
                nc.vector.tensor_copy(out=b_row, in_=bs)

            def load_weight(w_ext, wT):
                for r in range(KT):
                    ws = stage.tile([P, H], F32, tag="w_stage", name=f"ws_{r}")
                    wb = stage.tile([P, H], BF16, tag="w_stage_bf", name=f"wb_{r}")
                    nc.sync.dma_start(out=ws, in_=w_ext[r * P:(r + 1) * P, :])
                    nc.vector.tensor_copy(out=wb, in_=ws)
                    nc.scalar.dma_start_transpose(
                        out=wT[:, :, r * P:(r + 1) * P], in_=wb)

            load_weight(wq_ext, wqT)

            # ---------------- x load & transpose ----------------
            for st in range(NT):
                s0, ssz = _stile(st)
                xs = stage.tile([P, H], F32, tag="x_stage", name=f"xs_{st}")
                xb = stage.tile([P, H], BF16, tag="x_stage_bf", name=f"xb_{st}")
                if ssz < P:
                    nc.vector.memset(xb, 0.0)
                nc.sync.dma_start(out=xs[:ssz], in_=x_ext[s0:s0 + ssz, :])
                nc.vector.tensor_copy(out=xb[:ssz], in_=xs[:ssz])
                nc.scalar.dma_start_transpose(out=xT[:, :, s0:s0 + P], in_=xb)

            load_weight(wk_ext, wkT)

            # ---------------- sin/cos tables (i-tile aligned) ----------------
            cstg = stage.tile([P, NT, D], F32, tag="cos_stage")
            sstg = stage.tile([P, NT, D], F32, tag="sin_stage")
            nc.vector.memset(cstg, 1.0)
            nc.vector.memset(sstg, 0.0)
            for st in range(NT):
                if st == 0:
                    nc.sync.dma_start(out=cstg[PREFIX:P, 0, :],
                                      in_=cos_ext[0:P - PREFIX, :])
                    nc.sync.dma_start(out=sstg[PREFIX:P, 0, :],
                                      in_=sin_ext[0:P - PREFIX, :])
                else:
                    r0 = st * P - PREFIX
                    rsz = min(P, NROT - r0)
                    nc.sync.dma_start(out=cstg[:rsz, st, :],
                                      in_=cos_ext[r0:r0 + rsz, :])
                    nc.sync.dma_start(out=sstg[:rsz, st, :],
                                      in_=sin_ext[r0:r0 + rsz, :])
            nc.vector.tensor_copy(out=ccos, in_=cstg)
            nc.vector.tensor_copy(out=ssin, in_=sstg)
            # bake rotate_half sign: first-half sin columns negated
            nc.vector.tensor_scalar_mul(ssin[:, :, 0:32], ssin[:, :, 0:32], -1.0)

            # ---------------- q/k projection + RoPE ----------------
            def qk_proj(wT, dstT, bias_row):
                for st in range(NT):
                    s0, _ = _stile(st)
                    ps = ps_sc.tile([P, 1024], F32, tag="sc",
                                    name=f"qkps_{st}")[:, :H]
                    for kt in range(KT):
                        for (o, n) in _nchunks(H):
                            nc.tensor.matmul(
                                ps[:, o:o + n],
                                xT[:, kt, s0:s0 + P],
                                wT[:, kt, o:o + n],
                                start=(kt == 0),
                                stop=(kt == KT - 1 and bias_row is None))
                    if bias_row is not None:
                        for (o, n) in _nchunks(H):
                            nc.tensor.matmul(ps[:, o:o + n], ones_row,
                                             bias_row[:, o:o + n],
                                             start=False, stop=True)
                    # RoPE straight from psum (DVE), then DMA-transpose
                    psv = ps.rearrange("p (h d) -> p h d", d=D)
                    rot = rope.tile([P, NH, D], BF16, tag="rot", name=f"rot_{st}")
                    qr = rope.tile([P, NH, D], BF16, tag="qr", name=f"qr_{st}")
                    nc.vector.tensor_mul(rot[:, :, 0:32], psv[:, :, 32:64],
                                         _bcast_mid(ssin[:, st, 0:32], NH))
                    nc.vector.tensor_mul(rot[:, :, 32:64], psv[:, :, 0:32],
                                         _bcast_mid(ssin[:, st, 32:64], NH))
                    nc.vector.tensor_mul(qr, psv, _bcast_mid(ccos[:, st, :], NH))
                    nc.vector.tensor_add(qr, qr, rot)
                    nc.scalar.dma_start_transpose(
                        out=dstT[:, :, s0:s0 + P],
                        in_=qr.rearrange("p h d -> p (h d)"))

            qk_proj(wqT, qT, bq_row)
            load_weight(wv_ext, wvT)
            qk_proj(wkT, kT, None)
            load_weight(wp_ext, wpT)

            # ---------------- v projection ----------------
            for st in range(NT):
                s0, ssz = _stile(st)
                ps = ps_sc.tile([P, 1024], F32, tag="sc", name=f"vps_{st}")[:, :H]
                for kt in range(KT):
                    for (o, n) in _nchunks(H):
                        nc.tensor.matmul(ps[:ssz, o:o + n],
                                         xT[:, kt, s0:s0 + ssz],
                                         wvT[:, kt, o:o + n],
                                         start=(kt == 0), stop=False)
                for (o, n) in _nchunks(H):
                    nc.tensor.matmul(ps[:ssz, o:o + n], ones_row[:, :ssz],
                                     bv_row[:, o:o + n], start=False, stop=True)
                nc.scalar.copy(out=vsb[:ssz, st, :, 0:D],
                               in_=ps[:ssz, :].rearrange("p (h d) -> p h d", d=D))

        # ---------------- attention ----------------
        es_pool = ctx.enter_context(tc.tile_pool(name="es_pool", bufs=3))
        norm_pool = ctx.enter_context(tc.tile_pool(name="norm_pool", bufs=4))
        outst = ctx.enter_context(tc.tile_pool(name="outst", bufs=2))
        dram_pool = ctx.enter_context(
            tc.tile_pool(name="dram_pool", bufs=1, space="DRAM"))
        rs_scratch = dram_pool.tile([NH * len(ICH), SCR_W], F32)

        def out_proj(st):
            s0, ssz = _stile(st)
            pss = []
            for ci, (o, n) in enumerate(_nchunks(H)):
                ps = ps_pv.tile([P, 512], F32, tag="pv",
                                name=f"ops_{st}_{ci}")[:, :n]
                for kt in range(KT):
                    nc.tensor.matmul(ps[:ssz, :], ctxT[:, kt, s0:s0 + ssz],
                                     wpT[:, kt, o:o + n],
                                     start=(kt == 0), stop=False)
                nc.tensor.matmul(ps[:ssz, :], ones_row[:, :ssz],
                                 bp_row[:, o:o + n], start=False, stop=True)
                pss.append(ps)
            ot = outst.tile([P, H], F32, tag="ostage", name=f"ost_{st}")
            for ci, (o, n) in enumerate(_nchunks(H)):
                nc.scalar.copy(out=ot[:ssz, o:o + n], in_=pss[ci][:ssz, :])
            nc.sync.dma_start(out=out_ext[s0:s0 + ssz, :], in_=ot[:ssz])

        pending_out = []   # i-tiles whose out-proj is deferred one pt iter
        done_itiles = 0

        for c, (i0, ilen) in enumerate(ICH):
            for pt in range(KT):
                heads = (2 * pt, 2 * pt + 1)
                use_dve = pt in DVE_EXP_PTS
                pvs = [ps_pv.tile([P, 512], F32, tag="pv",
                                  name=f"pv_{c}_{pt}_{hh}")[:, :ilen]
                       for hh in range(2)]
                for jt in range(NT):
                    j0, jsz = _stile(jt)
                    sc = ps_sc.tile([P, 1024], F32, tag="sc",
                                    name=f"sc_{c}_{pt}_{jt}")
                    for hh in range(2):
                        hb = 64 * hh
                        nc.tensor.matmul(
                            sc[0:jsz, 512 * hh:512 * hh + ilen],
                            kT[hb:hb + 64, pt, j0:j0 + jsz],
                            qT[hb:hb + 64, pt, i0:i0 + ilen],
                            start=True, stop=True,
                            tile_position=(hb, 0))
                    es = es_pool.tile([P, 1024], BF16, tag="es",
                                      name=f"es_{c}_{pt}_{jt}")
                    if use_dve:
                        nc.scalar.activation(
                            out=es[0:jsz, 0:ilen], in_=sc[0:jsz, 0:ilen],
                            func=mybir.ActivationFunctionType.Exp,
                            scale=SCALING)
                        nc.vector.tensor_scalar(
                            out=es[0:jsz, 512:512 + ilen].bitcast(I16),
                            in0=sc[0:jsz, 512:512 + ilen],
                            scalar1=EXP_A, scalar2=EXP_B,
                            op0=mybir.AluOpType.mult, op1=mybir.AluOpType.add)
                    else:
                        # one ACT covering both heads (and, for the last
                        # chunk, the bank gap between them - stale psum
                        # there holds old bounded scores, exp of it is
                        # finite and the es gap columns are never read)
                        nc.scalar.activation(
                            out=es[0:jsz, 0:512 + ilen],
                            in_=sc[0:jsz, 0:512 + ilen],
                            func=mybir.ActivationFunctionType.Exp,
                            scale=SCALING)
                    for hh in range(2):
                        nc.tensor.matmul(
                            pvs[hh][0:D + 1, :],
                            vsb[0:jsz, jt, heads[hh], :],
                            es[0:jsz, 512 * hh:512 * hh + ilen],
                            start=(jt == 0), stop=(jt == NT - 1))
                # normalize -> ctxT
                for hh in range(2):
                    idx = heads[hh] * len(ICH) + c
                    dn = norm_pool.tile([1, SCR_W], F32, tag="dn",
                                        name=f"dn_{c}_{pt}_{hh}")
                    dr = norm_pool.tile([1, SCR_W], F32, tag="dr",
                                        name=f"dr_{c}_{pt}_{hh}")
                    nc.vector.tensor_copy(out=dn[:, :ilen],
                                          in_=pvs[hh][D:D + 1, :])
                    nc.vector.reciprocal_approx_fast(out=dr[:, :ilen],
                                                     in_=dn[:, :ilen])
                    nc.sync.dma_start(out=rs_scratch[idx:idx + 1, :ilen],
                                      in_=dr[:, :ilen])
                    bc = norm_pool.tile([D, SCR_W], F32, tag="bc",
                                        name=f"bc_{c}_{pt}_{hh}")[:, :ilen]
                    scr_row = rs_scratch[idx:idx + 1, :ilen]
                    bcast_src = bass.AP(
                        tensor=scr_row.tensor, offset=scr_row.offset,
                        ap=[[0, D]] + [list(a) for a in scr_row.ap[1:]])
                    nc.sync.dma_start(out=bc, in_=bcast_src)
                    nc.vector.tensor_mul(
                        ctxT[64 * hh:64 * hh + 64, pt, i0:i0 + ilen],
                        pvs[hh][0:D, :], bc)
                # deferred out-proj: emit the previous chunk's tiles here so
                # their matmuls sit behind this pt's attention work in the
                # PE queue (ctxT deps resolve while the PE streams scores)
                if pending_out and pt == 0:
                    for st in pending_out:
                        out_proj(st)
                    pending_out = []
            lim = (i0 + ilen) // P
            pending_out = list(range(done_itiles, lim))
            done_itiles = lim
        for st in range(done_itiles, NT):
            pending_out.append(st)
        for st in pending_out:
            out_proj(st)


_NC_CACHE = None


def get_nc():
    global _NC_CACHE
    if _NC_CACHE is None:
        nc = bacc.Bacc(None, target_bir_lowering=False, debug=False)
        _NC_CACHE = build_kernel(nc)
    return _NC_CACHE


def kernel(**inputs):
    from concourse.bass_utils import run_bass_kernel_spmd

    nc = get_nc()
    names = ["hidden_states", "sin", "cos", "Wq", "bq", "Wk", "Wv", "bv", "Wp", "bp"]
    arrs = {k: np.ascontiguousarray(np.asarray(inputs[k], dtype=np.float32))
            for k in names}
    in_maps = []
    for b in range(B):
        m = {k: arrs[k] for k in names if k != "hidden_states"}
        m["hidden_states"] = np.ascontiguousarray(arrs["hidden_states"][b])
        in_maps.append(m)
    res = run_bass_kernel_spmd(nc, in_maps, core_ids=list(range(B)))
    out = np.stack([res.results[b]["out"] for b in range(B)], axis=0)
    return out.astype(np.float32)


if __name__ == "__main__":
    nc = get_nc()
    print("built ok")


# revision 4
# speedup vs baseline: 1.4293x; 1.0714x over previous
"""Dinov3 self-attention Bass kernel for TRN2.

Sharding: data-parallel over batch. B=8 batch elements -> 8 NeuronCores,
one full attention per core, weights replicated. No collectives.

Per-core structure (matmuls bf16 x bf16 -> fp32 PSUM unless noted):
  The DMA-transpose xbar is a scarce serial resource (~5us per 128x768
  tile), so only x (11 tiles) and the sin/cos prep (2 tiles) use it.
  All four weights are transposed ON THE PE (f32 tensor.transpose into
  psum + DVE evict-cast to bf16) during the DMA-bound prologue.
  q/k projections produce qT/kT [o, s] DIRECTLY (lhsT = W^T, rhs = x^T)
  so q/k never need a transpose; q bias is a per-partition ACT bias at
  eviction.  RoPE in this layout pairs PARTITIONS: 4 partition-shift
  DMAs build rotate_half, sin/cos tables live transposed (cc2/ss2,
  prefix cols baked to 1/0, rotate sign baked into ss2 rows).
  v natural -> vsb[j, jt, h, 65] with a ones column per head (PV matmul
  computes ctx rows AND the softmax denominator in one M=65 matmul).
  Attention per (chunk, head-pair, jt): two K=64 score MMs run
  concurrently on disjoint PE row strips into two 1-bank psum tiles;
  even head exp on ACT, odd head exp on DVE via a one-op exp2 bit trick
  (tensor_scalar mult-add -> int16 bits == bf16 exp approximation).
  PV accumulates ctx_u^T over jt; denominators get reciprocal_approx
  then one DRAM bounce for the partition-broadcast; DVE mul -> ctxT.
  out[i, o] = ctxT^T @ WpT (+ ones x bp) -> fp32 -> DRAM, emission
  deferred one head-pair so out-proj MMs never head-block the PE queue.
"""

import contextlib
import sys

import numpy as np

sys.path.insert(0, "/opt/trn_rl_repo")

import concourse.bacc as bacc
import concourse.bass as bass
import concourse.tile as tile
from concourse import mybir
from concourse.masks import make_identity

S = 1374
H = 768
NH = 12
D = 64
NROT = 1369
PREFIX = S - NROT  # 5
B = 8

P = 128
NT = (S + P - 1) // P   # 11 s-tiles, last has 94 rows
KT = H // P             # 6 contraction blocks
SPAD = NT * P           # 1408
ICH = ((0, 512), (512, 512), (1024, 350))  # i-chunks
SCR_W = 512             # denominator scratch row width

# exp(z) ~ bf16_bits(round(z*log2e*128 + 128*(127-sigma))), z = s/8
EXP_A = 16.0 * 1.4426950408889634          # 128 * log2(e) / 8
EXP_B = 128.0 * (127.0 - 0.058)

F32 = mybir.dt.float32
BF16 = mybir.dt.bfloat16
I16 = mybir.dt.int16

SCALING = float(D) ** -0.5


def _stile(i):
    start = i * P
    return start, min(P, S - start)


def _nchunks(total, width=512):
    out, off = [], 0
    while off < total:
        n = min(width, total - off)
        out.append((off, n))
        off += n
    return out


def build_kernel(nc):
    x_ext = nc.declare_dram_parameter("hidden_states", [S, H], F32, isOutput=False)
    sin_ext = nc.declare_dram_parameter("sin", [NROT, D], F32, isOutput=False)
    cos_ext = nc.declare_dram_parameter("cos", [NROT, D], F32, isOutput=False)
    wq_ext = nc.declare_dram_parameter("Wq", [H, H], F32, isOutput=False)
    bq_ext = nc.declare_dram_parameter("bq", [H], F32, isOutput=False)
    wk_ext = nc.declare_dram_parameter("Wk", [H, H], F32, isOutput=False)
    wv_ext = nc.declare_dram_parameter("Wv", [H, H], F32, isOutput=False)
    bv_ext = nc.declare_dram_parameter("bv", [H], F32, isOutput=False)
    wp_ext = nc.declare_dram_parameter("Wp", [H, H], F32, isOutput=False)
    bp_ext = nc.declare_dram_parameter("bp", [H], F32, isOutput=False)
    out_ext = nc.declare_dram_parameter("out", [S, H], F32, isOutput=True)

    with tile.TileContext(nc) as tc:
        _body(tc, x_ext, sin_ext, cos_ext, wq_ext, bq_ext, wk_ext,
              wv_ext, bv_ext, wp_ext, bp_ext, out_ext)
    nc.compile()
    return nc


def _body(tc, x_ext, sin_ext, cos_ext, wq_ext, bq_ext, wk_ext, wv_ext,
          bv_ext, wp_ext, bp_ext, out_ext):
    nc = tc.nc

    with contextlib.ExitStack() as ctx:
        persist = ctx.enter_context(tc.tile_pool(name="persist", bufs=1))
        # single psum pool: 8 x [128, 512] f32 = all 8 banks
        pool8 = ctx.enter_context(tc.tile_pool(name="pool8", bufs=8, space="PSUM"))

        xT = persist.tile([P, KT, SPAD], BF16)
        wqT = persist.tile([P, KT, H], BF16)
        wkT = persist.tile([P, KT, H], BF16)
        wvT = persist.tile([P, KT, H], BF16)
        wpT = persist.tile([P, KT, H], BF16)
        qT = persist.tile([P, KT, SPAD], BF16)
        kT = persist.tile([P, KT, SPAD], BF16)
        ctxT = persist.tile([P, KT, SPAD], BF16)
        vsb = persist.tile([P, NT, NH, D + 1], BF16)
        cc2 = persist.tile([P, SPAD], BF16)   # cos^T stacked twice, prefix=1
        ss2 = persist.tile([P, SPAD], BF16)   # sin^T stacked, sign-baked, prefix=0
        bq_sb = persist.tile([P, KT], F32)
        bv_row = persist.tile([1, H], BF16)
        bp_row = persist.tile([1, H], BF16)
        ones_row = persist.tile([1, P], BF16)
        ident = persist.tile([P, P], F32)

        nc.vector.memset(ones_row, 1.0)
        nc.vector.memset(vsb[:, :, :, D:D + 1], 1.0)
        nc.vector.memset(ctxT[:, :, S:SPAD], 0.0)
        make_identity(nc, ident)

        # preload the exp table set so the first real exp doesn't stall
        with tc.tile_pool(name="warm", bufs=1) as warm:
            wtile = warm.tile([1, 2], F32)
            nc.vector.memset(wtile, 0.0)
            nc.scalar.activation(out=wtile[:, 1:2], in_=wtile[:, 0:1],
                                 func=mybir.ActivationFunctionType.Exp)

        with tc.tile_pool(name="stage", bufs=3) as stage, \
             tc.tile_pool(name="rope", bufs=2) as rope:

            # ---------------- biases ----------------
            nc.sync.dma_start(out=bq_sb,
                              in_=bq_ext.rearrange("(t p) -> p t", p=P))
            for b_ext, b_row in ((bv_ext, bv_row), (bp_ext, bp_row)):
                bs = stage.tile([1, H], F32, tag="bias_stage")
                nc.sync.dma_start(out=bs, in_=b_ext.rearrange("(a h) -> a h", a=1))
                nc.vector.tensor_copy(out=b_row, in_=bs)

            # ------------- weights: PE transpose, DVE evict-cast -------------
            def load_weight(w_ext, wT):
                for r in range(KT):
                    ws = stage.tile([P, H], F32, tag="w_stage", name=f"ws_{r}")
                    nc.sync.dma_start(out=ws, in_=w_ext[r * P:(r + 1) * P, :])
                    for g, cn in ((0, 4), (4, 2)):  # psum groups of 4 + 2 pieces
                        tp = pool8.tile([P, 512], F32, tag="ps",
                                        name=f"wt_{r}_{g}")
                        for k in range(cn):
                            c = g + k
                            nc.tensor.transpose(
                                tp[:, k * P:(k + 1) * P],
                                ws[:, c * P:(c + 1) * P], ident)
                        nc.vector.tensor_copy(
                            out=wT[:, g:g + cn, r * P:(r + 1) * P],
                            in_=tp[:, :cn * P].rearrange(
                                "p (c q) -> p c q", q=P))

            load_weight(wq_ext, wqT)
            load_weight(wk_ext, wkT)
            load_weight(wv_ext, wvT)
            load_weight(wp_ext, wpT)

            # ---------------- x load & transpose (xbar) ----------------
            def load_x(st):
                s0, ssz = _stile(st)
                xs = stage.tile([P, H], F32, tag="x_stage", name=f"xs_{st}")
                xb = stage.tile([P, H], BF16, tag="x_stage_bf", name=f"xb_{st}")
                if ssz < P:
                    nc.vector.memset(xb, 0.0)
                nc.sync.dma_start(out=xs[:ssz], in_=x_ext[s0:s0 + ssz, :])
                nc.vector.tensor_copy(out=xb[:ssz], in_=xs[:ssz])
                nc.scalar.dma_start_transpose(out=xT[:, :, s0:s0 + P], in_=xb)

            for st in range(4):
                load_x(st)

            # ------------- sin/cos -> transposed tables cc2/ss2 -------------
            n_rtile = (NROT + P - 1) // P
            nc.vector.memset(cc2, 0.0)
            nc.vector.memset(cc2[:, 0:PREFIX], 1.0)
            nc.vector.memset(ss2, 0.0)
            for src_ext, dstT in ((cos_ext, cc2), (sin_ext, ss2)):
                cst = stage.tile([P, SPAD], F32, tag="cs_stage")
                csb = stage.tile([P, SPAD], BF16, tag="cs_stage_bf")
                csT3 = stage.tile([P, n_rtile, P], BF16, tag="cs_T3")
                nc.vector.memset(csb, 0.0)
                for i in range(n_rtile):
                    r0 = i * P
                    rsz = min(P, NROT - r0)
                    nc.sync.dma_start(out=cst[:rsz, i * P:i * P + D],
                                      in_=src_ext[r0:r0 + rsz, :])
                    nc.vector.tensor_copy(out=csb[:rsz, i * P:i * P + D],
                                          in_=cst[:rsz, i * P:i * P + D])
                nc.scalar.dma_start_transpose(out=csT3, in_=csb)
                for i in range(n_rtile):
                    r0 = i * P
                    rsz = min(P, NROT - r0)
                    for half in range(2):
                        nc.vector.tensor_copy(
                            out=dstT[64 * half:64 * half + 64,
                                     PREFIX + r0:PREFIX + r0 + rsz],
                            in_=csT3[0:D, i, :rsz])
            for base in (0, 64):  # bake rotate_half sign
                nc.vector.tensor_scalar_mul(ss2[base:base + 32, :],
                                            ss2[base:base + 32, :], -1.0)

            for st in range(4, NT):
                load_x(st)

            # ---------------- q/k projection (transposed out) + RoPE --------
            def qk_proj(wT, dstT, with_bias):
                for ot in range(KT):
                    pss = [pool8.tile([P, 512], F32, tag="ps",
                                      name=f"qk_{ot}_{ci}")[:, :n]
                           for ci, (o, n) in enumerate(_nchunks(S))]
                    for kt in range(KT):
                        for ci, (i0, n) in enumerate(_nchunks(S)):
                            nc.tensor.matmul(
                                pss[ci],
                                wT[:, kt, ot * P:(ot + 1) * P],
                                xT[:, kt, i0:i0 + n],
                                start=(kt == 0), stop=(kt == KT - 1))
                    qb = rope.tile([P, SPAD], BF16, tag="qb", name=f"qb_{ot}")
                    for ci, (i0, n) in enumerate(_nchunks(S)):
                        if with_bias:
                            nc.scalar.add(qb[:, i0:i0 + n], pss[ci],
                                          bq_sb[:, ot:ot + 1])
                        else:
                            nc.scalar.copy(out=qb[:, i0:i0 + n], in_=pss[ci])
                    rot = rope.tile([P, SPAD], BF16, tag="rot", name=f"rot_{ot}")
                    for (dst0, src0) in ((0, 32), (32, 0), (64, 96), (96, 64)):
                        nc.sync.dma_start(out=rot[dst0:dst0 + 32, 0:S],
                                          in_=qb[src0:src0 + 32, 0:S])
                    nc.vector.tensor_mul(dstT[:, ot, 0:S], qb[:, 0:S],
                                         cc2[:, 0:S])
                    nc.vector.tensor_mul(rot[:, 0:S], rot[:, 0:S], ss2[:, 0:S])
                    nc.vector.tensor_add(dstT[:, ot, 0:S], dstT[:, ot, 0:S],
                                         rot[:, 0:S])

            qk_proj(wqT, qT, True)
            qk_proj(wkT, kT, False)

            # ---------------- v projection (natural out) ----------------
            for st in range(NT):
                s0, ssz = _stile(st)
                pss = []
                for ci, (o, n) in enumerate(_nchunks(H)):
                    ps = pool8.tile([P, 512], F32, tag="ps",
                                    name=f"v_{st}_{ci}")[:, :n]
                    for kt in range(KT):
                        nc.tensor.matmul(ps[:ssz], xT[:, kt, s0:s0 + ssz],
                                         wvT[:, kt, o:o + n],
                                         start=(kt == 0), stop=False)
                    nc.tensor.matmul(ps[:ssz], ones_row[:, :ssz],
                                     bv_row[:, o:o + n], start=False, stop=True)
                    pss.append(ps)
                for ci, (o, n) in enumerate(_nchunks(H)):
                    nc.scalar.copy(
                        out=vsb[:ssz, st, o // D:(o + n) // D, 0:D],
                        in_=pss[ci][:ssz].rearrange("p (h d) -> p h d", d=D))

        # ---------------- attention ----------------
        es_pool = ctx.enter_context(tc.tile_pool(name="es_pool", bufs=4))
        norm_pool = ctx.enter_context(tc.tile_pool(name="norm_pool", bufs=4))
        outst = ctx.enter_context(tc.tile_pool(name="outst", bufs=2))
        dram_pool = ctx.enter_context(
            tc.tile_pool(name="dram_pool", bufs=1, space="DRAM"))
        rs_scratch = dram_pool.tile([NH * len(ICH), SCR_W], F32)

        def out_proj(st):
            s0, ssz = _stile(st)
            pss = []
            for ci, (o, n) in enumerate(_nchunks(H)):
                ps = pool8.tile([P, 512], F32, tag="ps",
                                name=f"ops_{st}_{ci}")[:, :n]
                for kt in range(KT):
                    nc.tensor.matmul(ps[:ssz], ctxT[:, kt, s0:s0 + ssz],
                                     wpT[:, kt, o:o + n],
                                     start=(kt == 0), stop=False)
                nc.tensor.matmul(ps[:ssz], ones_row[:, :ssz],
                                 bp_row[:, o:o + n], start=False, stop=True)
                pss.append(ps)
            ot = outst.tile([P, H], F32, tag="ostage", name=f"ost_{st}")
            for ci, (o, n) in enumerate(_nchunks(H)):
                nc.scalar.copy(out=ot[:ssz, o:o + n], in_=pss[ci][:ssz])
            nc.sync.dma_start(out=out_ext[s0:s0 + ssz, :], in_=ot[:ssz])

        pending_out = []
        done_itiles = 0

        for c, (i0, ilen) in enumerate(ICH):
            for pt in range(KT):
                heads = (2 * pt, 2 * pt + 1)
                pvs = [pool8.tile([P, 512], F32, tag="ps",
                                  name=f"pv_{c}_{pt}_{hh}")[:, :ilen]
                       for hh in range(2)]
                for jt in range(NT):
                    j0, jsz = _stile(jt)
                    scs = [pool8.tile([P, 512], F32, tag="ps",
                                      name=f"sc_{c}_{pt}_{jt}_{hh}")
                           for hh in range(2)]
                    for hh in range(2):
                        hb = 64 * hh
                        nc.tensor.matmul(
                            scs[hh][0:jsz, :ilen],
                            kT[hb:hb + 64, pt, j0:j0 + jsz],
                            qT[hb:hb + 64, pt, i0:i0 + ilen],
                            start=True, stop=True,
                            tile_position=(hb, 0))
                    es = es_pool.tile([P, 1024], BF16, tag="es",
                                      name=f"es_{c}_{pt}_{jt}")
                    # even head: exact exp on ACT; odd head: DVE exp2 bit trick
                    nc.scalar.activation(
                        out=es[0:jsz, 0:ilen], in_=scs[0][0:jsz, :ilen],
                        func=mybir.ActivationFunctionType.Exp, scale=SCALING)
                    nc.vector.tensor_scalar(
                        out=es[0:jsz, 512:512 + ilen].bitcast(I16),
                        in0=scs[1][0:jsz, :ilen],
                        scalar1=EXP_A, scalar2=EXP_B,
                        op0=mybir.AluOpType.mult, op1=mybir.AluOpType.add)
                    for hh in range(2):
                        nc.tensor.matmul(
                            pvs[hh][0:D + 1, :],
                            vsb[0:jsz, jt, heads[hh], :],
                            es[0:jsz, 512 * hh:512 * hh + ilen],
                            start=(jt == 0), stop=(jt == NT - 1))
                # normalize -> ctxT
                for hh in range(2):
                    idx = heads[hh] * len(ICH) + c
                    dn = norm_pool.tile([1, SCR_W], F32, tag="dn",
                                        name=f"dn_{c}_{pt}_{hh}")
                    dr = norm_pool.tile([1, SCR_W], F32, tag="dr",
                                        name=f"dr_{c}_{pt}_{hh}")
                    nc.vector.tensor_copy(out=dn[:, :ilen],
                                          in_=pvs[hh][D:D + 1, :])
                    nc.vector.reciprocal_approx_fast(out=dr[:, :ilen],
                                                     in_=dn[:, :ilen])
                    nc.sync.dma_start(out=rs_scratch[idx:idx + 1, :ilen],
                                      in_=dr[:, :ilen])
                    bc = norm_pool.tile([D, SCR_W], F32, tag="bc",
                                        name=f"bc_{c}_{pt}_{hh}")[:, :ilen]
                    scr_row = rs_scratch[idx:idx + 1, :ilen]
                    bcast_src = bass.AP(
                        tensor=scr_row.tensor, offset=scr_row.offset,
                        ap=[[0, D]] + [list(a) for a in scr_row.ap[1:]])
                    nc.sync.dma_start(out=bc, in_=bcast_src)
                    nc.vector.tensor_mul(
                        ctxT[64 * hh:64 * hh + 64, pt, i0:i0 + ilen],
                        pvs[hh][0:D, :], bc)
                # deferred out-proj: previous chunk's tiles ride behind this
                # pt's attention matmuls in the PE queue
                if pending_out and pt == 0:
                    for st in pending_out:
                        out_proj(st)
                    pending_out = []
            lim = (i0 + ilen) // P
            pending_out = list(range(done_itiles, lim))
            done_itiles = lim
        for st in range(done_itiles, NT):
            pending_out.append(st)
        for st in pending_out:
            out_proj(st)


_NC_CACHE = None


def get_nc():
    global _NC_CACHE
    if _NC_CACHE is None:
        nc = bacc.Bacc(None, target_bir_lowering=False, debug=False)
        _NC_CACHE = build_kernel(nc)
    return _NC_CACHE


def kernel(**inputs):
    from concourse.bass_utils import run_bass_kernel_spmd

    nc = get_nc()
    names = ["hidden_states", "sin", "cos", "Wq", "bq", "Wk", "Wv", "bv", "Wp", "bp"]
    arrs = {k: np.ascontiguousarray(np.asarray(inputs[k], dtype=np.float32))
            for k in names}
    in_maps = []
    for b in range(B):
        m = {k: arrs[k] for k in names if k != "hidden_states"}
        m["hidden_states"] = np.ascontiguousarray(arrs["hidden_states"][b])
        in_maps.append(m)
    res = run_bass_kernel_spmd(nc, in_maps, core_ids=list(range(B)))
    out = np.stack([res.results[b]["out"] for b in range(B)], axis=0)
    return out.astype(np.float32)


if __name__ == "__main__":
    nc = get_nc()
    print("built ok")


# revision 8
# speedup vs baseline: 1.9831x; 1.3875x over previous
"""Dinov3 self-attention Bass kernel for TRN2.

Sharding: data-parallel over batch. B=8 batch elements -> 8 NeuronCores,
one full attention per core, weights replicated. No collectives.

Per-core structure (matmuls bf16 x bf16 -> fp32 PSUM unless noted):
  The DMA-transpose xbar is a scarce serial resource (~5us per 128x768
  tile), so only x (11 tiles) and the sin/cos prep (2 tiles) use it.
  All four weights are transposed ON THE PE (f32 tensor.transpose into
  psum + DVE evict-cast to bf16) during the DMA-bound prologue.
  q/k projections produce qT/kT [o, s] DIRECTLY (lhsT = W^T, rhs = x^T)
  so q/k never need a transpose; q bias is a per-partition ACT bias at
  eviction.  RoPE in this layout pairs PARTITIONS: 4 partition-shift
  DMAs build rotate_half, sin/cos tables live transposed (cc2/ss2,
  prefix cols baked to 1/0, rotate sign baked into ss2 rows).
  v natural -> vsb[j, jt, h, 65] with a ones column per head (PV matmul
  computes ctx rows AND the softmax denominator in one M=65 matmul).
  Attention per (chunk, head-pair, jt): two K=64 score MMs run
  concurrently on disjoint PE row strips into two 1-bank psum tiles;
  even head exp on ACT, odd head exp on DVE via a one-op exp2 bit trick
  (tensor_scalar mult-add -> int16 bits == bf16 exp approximation).
  PV accumulates ctx_u^T over jt; denominators get reciprocal_approx
  then one DRAM bounce for the partition-broadcast; DVE mul -> ctxT.
  out[i, o] = ctxT^T @ WpT (+ ones x bp) -> fp32 -> DRAM, emission
  deferred one head-pair so out-proj MMs never head-block the PE queue.
"""

import contextlib
import sys

import numpy as np

sys.path.insert(0, "/opt/trn_rl_repo")

import concourse.bacc as bacc
import concourse.bass as bass
import concourse.tile as tile
from concourse import mybir
from concourse.masks import make_identity

S = 1374
H = 768
NH = 12
D = 64
NROT = 1369
PREFIX = S - NROT  # 5
B = 8

P = 128
NT = (S + P - 1) // P   # 11 s-tiles, last has 94 rows
KT = H // P             # 6 contraction blocks
SPAD = NT * P           # 1408
ICH = ((0, 512), (512, 512), (1024, 350))  # i-chunks
SCR_W = 512             # denominator scratch row width

# exp(z) ~ bf16_bits(round(z*log2e*128 + 128*(127-sigma))), z = s/8
EXP_A = 16.0 * 1.4426950408889634          # 128 * log2(e) / 8
EXP_B = 128.0 * (127.0 - 0.058)

F32 = mybir.dt.float32
BF16 = mybir.dt.bfloat16
I16 = mybir.dt.int16

SCALING = float(D) ** -0.5


def _stile(i):
    start = i * P
    return start, min(P, S - start)


def _nchunks(total, width=512):
    out, off = [], 0
    while off < total:
        n = min(width, total - off)
        out.append((off, n))
        off += n
    return out


def build_kernel(nc):
    x_ext = nc.declare_dram_parameter("hidden_states", [S, H], F32, isOutput=False)
    sin_ext = nc.declare_dram_parameter("sin", [NROT, D], F32, isOutput=False)
    cos_ext = nc.declare_dram_parameter("cos", [NROT, D], F32, isOutput=False)
    wq_ext = nc.declare_dram_parameter("Wq", [H, H], F32, isOutput=False)
    bq_ext = nc.declare_dram_parameter("bq", [H], F32, isOutput=False)
    wk_ext = nc.declare_dram_parameter("Wk", [H, H], F32, isOutput=False)
    wv_ext = nc.declare_dram_parameter("Wv", [H, H], F32, isOutput=False)
    bv_ext = nc.declare_dram_parameter("bv", [H], F32, isOutput=False)
    wp_ext = nc.declare_dram_parameter("Wp", [H, H], F32, isOutput=False)
    bp_ext = nc.declare_dram_parameter("bp", [H], F32, isOutput=False)
    out_ext = nc.declare_dram_parameter("out", [S, H], F32, isOutput=True)

    with tile.TileContext(nc) as tc:
        _body(tc, x_ext, sin_ext, cos_ext, wq_ext, bq_ext, wk_ext,
              wv_ext, bv_ext, wp_ext, bp_ext, out_ext)
    nc.compile()
    return nc


def _body(tc, x_ext, sin_ext, cos_ext, wq_ext, bq_ext, wk_ext, wv_ext,
          bv_ext, wp_ext, bp_ext, out_ext):
    nc = tc.nc

    with contextlib.ExitStack() as ctx:
        persist = ctx.enter_context(tc.tile_pool(name="persist", bufs=1))
        # single psum pool: 8 x [128, 512] f32 = all 8 banks
        pool8 = ctx.enter_context(tc.tile_pool(name="pool8", bufs=8, space="PSUM"))

        xT = persist.tile([P, KT, SPAD], BF16)
        wqT = persist.tile([P, KT, H], BF16)
        wkT = persist.tile([P, KT, H], BF16)
        wvT = persist.tile([P, KT, H], BF16)
        wpT = persist.tile([P, KT, H], BF16)
        qT = persist.tile([P, KT, SPAD], BF16)
        kT = persist.tile([P, KT, SPAD], BF16)
        ctxT = persist.tile([P, KT, SPAD], BF16)
        vsb = persist.tile([P, NT, NH, D + 1], BF16)
        cc2 = persist.tile([P, SPAD], BF16)   # cos^T stacked twice, prefix=1
        ss2 = persist.tile([P, SPAD], BF16)   # sin^T stacked, sign-baked, prefix=0
        bq_sb = persist.tile([P, KT], F32)
        bv_row = persist.tile([1, H], BF16)
        bp_row = persist.tile([1, H], BF16)
        ones_row = persist.tile([1, P], BF16)
        ident = persist.tile([P, P], F32)

        nc.vector.memset(ones_row, 1.0)
        nc.vector.memset(vsb[:, :, :, D:D + 1], 1.0)
        nc.vector.memset(ctxT[:, :, S:SPAD], 0.0)
        make_identity(nc, ident)

        # preload the exp table set so the first real exp doesn't stall
        with tc.tile_pool(name="warm", bufs=1) as warm:
            wtile = warm.tile([1, 2], F32)
            nc.vector.memset(wtile, 0.0)
            nc.scalar.activation(out=wtile[:, 1:2], in_=wtile[:, 0:1],
                                 func=mybir.ActivationFunctionType.Exp)

        with tc.tile_pool(name="stage", bufs=3) as stage, \
             tc.tile_pool(name="rope", bufs=2) as rope:

            # ---------------- biases ----------------
            nc.sync.dma_start(out=bq_sb,
                              in_=bq_ext.rearrange("(t p) -> p t", p=P))
            for b_ext, b_row in ((bv_ext, bv_row), (bp_ext, bp_row)):
                bs = stage.tile([1, H], F32, tag="bias_stage")
                nc.sync.dma_start(out=bs, in_=b_ext.rearrange("(a h) -> a h", a=1))
                nc.vector.tensor_copy(out=b_row, in_=bs)

            # ------------- weights: PE transpose, DVE evict-cast -------------
            def load_weight(w_ext, wT):
                for r in range(KT):
                    ws = stage.tile([P, H], F32, tag="w_stage", name=f"ws_{r}")
                    nc.sync.dma_start(out=ws, in_=w_ext[r * P:(r + 1) * P, :])
                    for g, cn in ((0, 4), (4, 2)):  # psum groups of 4 + 2 pieces
                        tp = pool8.tile([P, 512], F32, tag="ps",
                                        name=f"wt_{r}_{g}")
                        for k in range(cn):
                            c = g + k
                            nc.tensor.transpose(
                                tp[:, k * P:(k + 1) * P],
                                ws[:, c * P:(c + 1) * P], ident)
                        nc.vector.tensor_copy(
                            out=wT[:, g:g + cn, r * P:(r + 1) * P],
                            in_=tp[:, :cn * P].rearrange(
                                "p (c q) -> p c q", q=P))

            load_weight(wq_ext, wqT)
            load_weight(wk_ext, wkT)
            load_weight(wv_ext, wvT)
            load_weight(wp_ext, wpT)

            # ------------- x: PE transpose from the f32 stage -------------
            def load_x(st):
                s0, ssz = _stile(st)
                xs = stage.tile([P, H], F32, tag="x_stage", name=f"xs_{st}")
                if ssz < P:
                    nc.vector.memset(xs, 0.0)
                nc.sync.dma_start(out=xs[:ssz], in_=x_ext[s0:s0 + ssz, :])
                for g, cn in ((0, 4), (4, 2)):
                    tp = pool8.tile([P, 512], F32, tag="ps",
                                    name=f"xt_{st}_{g}")
                    for k in range(cn):
                        c = g + k
                        nc.tensor.transpose(tp[:, k * P:(k + 1) * P],
                                            xs[:, c * P:(c + 1) * P], ident)
                    nc.vector.tensor_copy(
                        out=xT[:, g:g + cn, s0:s0 + P],
                        in_=tp[:, :cn * P].rearrange("p (c q) -> p c q", q=P))

            for st in range(NT):
                load_x(st)

            # ------------- sin/cos -> transposed tables cc2/ss2 -------------
            # (PE transpose: [rsz, 64] table tile -> [64, rsz] psum)
            n_rtile = (NROT + P - 1) // P
            nc.vector.memset(cc2, 0.0)
            nc.vector.memset(cc2[:, 0:PREFIX], 1.0)
            nc.vector.memset(ss2, 0.0)
            for src_ext, dstT in ((cos_ext, cc2), (sin_ext, ss2)):
                cst = stage.tile([P, n_rtile, D], F32, tag="cs_stage")
                nc.vector.memset(cst, 0.0)
                for i in range(n_rtile):
                    r0 = i * P
                    rsz = min(P, NROT - r0)
                    nc.sync.dma_start(out=cst[:rsz, i, :],
                                      in_=src_ext[r0:r0 + rsz, :])
                for g in range(0, n_rtile, 4):
                    cn = min(4, n_rtile - g)
                    width = min(cn * P, NROT - g * P)
                    tp = pool8.tile([P, 512], F32, tag="ps",
                                    name=f"cst_{g}")
                    for k in range(cn):
                        nc.tensor.transpose(tp[0:D, k * P:(k + 1) * P],
                                            cst[:, g + k, :], ident)
                    for half in range(2):
                        nc.vector.tensor_copy(
                            out=dstT[64 * half:64 * half + 64,
                                     PREFIX + g * P:PREFIX + g * P + width],
                            in_=tp[0:D, :width])
            for base in (0, 64):  # bake rotate_half sign
                nc.vector.tensor_scalar_mul(ss2[base:base + 32, :],
                                            ss2[base:base + 32, :], -1.0)

            # ---------------- q/k projection (transposed out) + RoPE --------
            def qk_proj(wT, dstT, with_bias):
                for ot in range(KT):
                    pss = [pool8.tile([P, 512], F32, tag="ps",
                                      name=f"qk_{ot}_{ci}")[:, :n]
                           for ci, (o, n) in enumerate(_nchunks(S))]
                    for kt in range(KT):
                        for ci, (i0, n) in enumerate(_nchunks(S)):
                            nc.tensor.matmul(
                                pss[ci],
                                wT[:, kt, ot * P:(ot + 1) * P],
                                xT[:, kt, i0:i0 + n],
                                start=(kt == 0), stop=(kt == KT - 1))
                    qb = rope.tile([P, SPAD], BF16, tag="qb", name=f"qb_{ot}")
                    for ci, (i0, n) in enumerate(_nchunks(S)):
                        if with_bias:
                            nc.scalar.add(qb[:, i0:i0 + n], pss[ci],
                                          bq_sb[:, ot:ot + 1])
                        else:
                            nc.scalar.copy(out=qb[:, i0:i0 + n], in_=pss[ci])
                    rot = rope.tile([P, SPAD], BF16, tag="rot", name=f"rot_{ot}")
                    for (dst0, src0) in ((0, 32), (32, 0), (64, 96), (96, 64)):
                        nc.sync.dma_start(out=rot[dst0:dst0 + 32, 0:S],
                                          in_=qb[src0:src0 + 32, 0:S])
                    nc.vector.tensor_mul(dstT[:, ot, 0:S], qb[:, 0:S],
                                         cc2[:, 0:S])
                    nc.vector.tensor_mul(rot[:, 0:S], rot[:, 0:S], ss2[:, 0:S])
                    nc.vector.tensor_add(dstT[:, ot, 0:S], dstT[:, ot, 0:S],
                                         rot[:, 0:S])

            qk_proj(wqT, qT, True)
            qk_proj(wkT, kT, False)

            # ---------------- v projection (natural out) ----------------
            for st in range(NT):
                s0, ssz = _stile(st)
                pss = []
                for ci, (o, n) in enumerate(_nchunks(H)):
                    ps = pool8.tile([P, 512], F32, tag="ps",
                                    name=f"v_{st}_{ci}")[:, :n]
                    for kt in range(KT):
                        nc.tensor.matmul(ps[:ssz], xT[:, kt, s0:s0 + ssz],
                                         wvT[:, kt, o:o + n],
                                         start=(kt == 0), stop=False)
                    nc.tensor.matmul(ps[:ssz], ones_row[:, :ssz],
                                     bv_row[:, o:o + n], start=False, stop=True)
                    pss.append(ps)
                for ci, (o, n) in enumerate(_nchunks(H)):
                    nc.scalar.copy(
                        out=vsb[:ssz, st, o // D:(o + n) // D, 0:D],
                        in_=pss[ci][:ssz].rearrange("p (h d) -> p h d", d=D))

        # ---------------- attention ----------------
        es_pool = ctx.enter_context(tc.tile_pool(name="es_pool", bufs=4))
        norm_pool = ctx.enter_context(tc.tile_pool(name="norm_pool", bufs=4))
        outst = ctx.enter_context(tc.tile_pool(name="outst", bufs=2))
        dram_pool = ctx.enter_context(
            tc.tile_pool(name="dram_pool", bufs=1, space="DRAM"))
        rs_scratch = dram_pool.tile([NH * len(ICH), SCR_W], F32)

        def out_proj(st):
            s0, ssz = _stile(st)
            pss = []
            for ci, (o, n) in enumerate(_nchunks(H)):
                ps = pool8.tile([P, 512], F32, tag="ps",
                                name=f"ops_{st}_{ci}")[:, :n]
                for kt in range(KT):
                    nc.tensor.matmul(ps[:ssz], ctxT[:, kt, s0:s0 + ssz],
                                     wpT[:, kt, o:o + n],
                                     start=(kt == 0), stop=False)
                nc.tensor.matmul(ps[:ssz], ones_row[:, :ssz],
                                 bp_row[:, o:o + n], start=False, stop=True)
                pss.append(ps)
            ot = outst.tile([P, H], F32, tag="ostage", name=f"ost_{st}")
            for ci, (o, n) in enumerate(_nchunks(H)):
                nc.scalar.copy(out=ot[:ssz, o:o + n], in_=pss[ci][:ssz])
            nc.sync.dma_start(out=out_ext[s0:s0 + ssz, :], in_=ot[:ssz])

        def norm(c, pt, pvs):
            i0, ilen = ICH[c]
            for hh in range(2):
                h = 2 * pt + hh
                idx = h * len(ICH) + c
                dn = norm_pool.tile([1, SCR_W], F32, tag="dn",
                                    name=f"dn_{c}_{pt}_{hh}")
                dr = norm_pool.tile([1, SCR_W], F32, tag="dr",
                                    name=f"dr_{c}_{pt}_{hh}")
                nc.scalar.copy(out=dn[:, :ilen], in_=pvs[hh][D:D + 1, :])
                nc.vector.reciprocal_approx_fast(out=dr[:, :ilen],
                                                 in_=dn[:, :ilen])
                nc.sync.dma_start(out=rs_scratch[idx:idx + 1, :ilen],
                                  in_=dr[:, :ilen])
                bc = norm_pool.tile([D, SCR_W], F32, tag="bc",
                                    name=f"bc_{c}_{pt}_{hh}")[:, :ilen]
                scr_row = rs_scratch[idx:idx + 1, :ilen]
                bcast_src = bass.AP(
                    tensor=scr_row.tensor, offset=scr_row.offset,
                    ap=[[0, D]] + [list(a) for a in scr_row.ap[1:]])
                nc.sync.dma_start(out=bc, in_=bcast_src)
                nc.vector.tensor_mul(
                    ctxT[64 * hh:64 * hh + 64, pt, i0:i0 + ilen],
                    pvs[hh][0:D, :], bc)

        pending_out = []
        pending_norm = None   # (c, pt, pvs) whose normalize is deferred
        done_itiles = 0

        for c, (i0, ilen) in enumerate(ICH):
            for pt in range(KT):
                heads = (2 * pt, 2 * pt + 1)
                pvs = [pool8.tile([P, 512], F32, tag="ps",
                                  name=f"pv_{c}_{pt}_{hh}")[:, :ilen]
                       for hh in range(2)]
                for jt in range(NT):
                    j0, jsz = _stile(jt)
                    scs = [pool8.tile([P, 512], F32, tag="ps",
                                      name=f"sc_{c}_{pt}_{jt}_{hh}")
                           for hh in range(2)]
                    for hh in range(2):
                        hb = 64 * hh
                        nc.tensor.matmul(
                            scs[hh][0:jsz, :ilen],
                            kT[hb:hb + 64, pt, j0:j0 + jsz],
                            qT[hb:hb + 64, pt, i0:i0 + ilen],
                            start=True, stop=True,
                            tile_position=(hb, 0))
                    es = es_pool.tile([P, 1024], BF16, tag="es",
                                      name=f"es_{c}_{pt}_{jt}")
                    # even head: exact exp on ACT; odd head: DVE exp2 bit trick
                    nc.scalar.activation(
                        out=es[0:jsz, 0:ilen], in_=scs[0][0:jsz, :ilen],
                        func=mybir.ActivationFunctionType.Exp, scale=SCALING)
                    nc.vector.tensor_scalar(
                        out=es[0:jsz, 512:512 + ilen].bitcast(I16),
                        in0=scs[1][0:jsz, :ilen],
                        scalar1=EXP_A, scalar2=EXP_B,
                        op0=mybir.AluOpType.mult, op1=mybir.AluOpType.add)
                    for hh in range(2):
                        nc.tensor.matmul(
                            pvs[hh][0:D + 1, :],
                            vsb[0:jsz, jt, heads[hh], :],
                            es[0:jsz, 512 * hh:512 * hh + ilen],
                            start=(jt == 0), stop=(jt == NT - 1))
                    # deferred normalize after the first jt of the NEXT
                    # head-pair: its DMA round-trips overlap attention
                    # instead of head-blocking the DVE queue
                    if jt == 0 and pending_norm is not None:
                        norm(*pending_norm)
                        pending_norm = None
                    # deferred out-proj similarly rides behind jt=1
                    if jt == 1 and pending_out:
                        for st in pending_out:
                            out_proj(st)
                        pending_out = []
                pending_norm = (c, pt, pvs)
            if c + 1 == len(ICH):
                continue
            lim = (i0 + ilen) // P
            pending_out = list(range(done_itiles, lim))
            done_itiles = lim
        norm(*pending_norm)
        for st in range(done_itiles, NT):
            out_proj(st)


_NC_CACHE = None


def get_nc():
    global _NC_CACHE
    if _NC_CACHE is None:
        nc = bacc.Bacc(None, target_bir_lowering=False, debug=False)
        _NC_CACHE = build_kernel(nc)
    return _NC_CACHE


def kernel(**inputs):
    from concourse.bass_utils import run_bass_kernel_spmd

    nc = get_nc()
    names = ["hidden_states", "sin", "cos", "Wq", "bq", "Wk", "Wv", "bv", "Wp", "bp"]
    arrs = {k: np.ascontiguousarray(np.asarray(inputs[k], dtype=np.float32))
            for k in names}
    in_maps = []
    for b in range(B):
        m = {k: arrs[k] for k in names if k != "hidden_states"}
        m["hidden_states"] = np.ascontiguousarray(arrs["hidden_states"][b])
        in_maps.append(m)
    res = run_bass_kernel_spmd(nc, in_maps, core_ids=list(range(B)))
    out = np.stack([res.results[b]["out"] for b in range(B)], axis=0)
    return out.astype(np.float32)


if __name__ == "__main__":
    nc = get_nc()
    print("built ok")
